# revision 28
# baseline (speedup 1.0000x reference)
"""Trainium2 Bass kernel for the MEGNet EdgeModel MLP.

Computes out = relu(relu(concat([src, dest, edge_attr, u[batch]]) @ W1 + b1) @ W2 + b2)
for 500k edges, sharded data-parallel over edges across 8 NeuronCores.

Device-side layout is feature-major (features on SBUF partitions, edges on the
free dim): the PE contracts over partitions with small stationary weights and
512-edge column blocks as the moving operand.

All matmuls run in bf16 with exact hi/lo splitting (x = x_hi + x_lo, each bf16;
W likewise), keeping full fp32-level precision while streaming at bf16 rates
(measured 4.2x faster than fp32 matmuls on this part):

    x@W ~= x_hi@W_hi + x_lo@W_hi + x_hi@W_lo      (dropped x_lo@W_lo ~ 2^-18)

src/dest pairs are packed on 128 partitions so one K=128 matmul covers both.

The u-gather term uses a table trick: v = u @ W1[192:256] is computed once
on-device in fp32, split into exact bf16 hi/lo halves packed as [1024, 128]
bf16 rows in DRAM. dma_gather(transpose=True) gathers per-edge rows directly
into feature-major layout (partitions 0..63 = v_hi, 64..127 = v_lo) and a
stacked-identity bf16 matmul adds v_hi + v_lo into the same PSUM accumulation.

Host side only reshapes/transposes/splits for sharding and converts batch
indices to the int16 wrapped layout dma_gather expects.
"""

import sys

import numpy as np

sys.path.insert(0, "/opt/trn_rl_repo")

import ml_dtypes  # noqa: E402

import concourse.bacc as bacc  # noqa: E402
import concourse.tile as tile  # noqa: E402
from concourse import mybir  # noqa: E402
from concourse.bass_utils import run_bass_kernel_spmd  # noqa: E402

N_EDGES = 500_000
D = 64
N_GRAPHS = 1024
N_CORES = 8

TS = 512  # edge tile (matmul moving free dim)
EC = 63488  # per-core padded edge count: 15 x 4096 + 1 x 2048
CHUNKS = [4096] * 15 + [2048]
assert sum(CHUNKS) == EC

F32 = mybir.dt.float32
BF16 = mybir.dt.bfloat16
I16 = mybir.dt.int16
BF16_NP = ml_dtypes.bfloat16

TRACE = False
LAST_RESULTS = None


def build_nc():
    nc = bacc.Bacc("TRN2")

    # packed bf16 activations, feature-major:
    #   p1 = [srcT_hi ; destT_hi], p2 = [srcT_lo ; destT_lo], p3 = [eaT_hi ; eaT_lo]
    p1 = nc.declare_dram_parameter("p1", [128, EC], BF16, isOutput=False)
    p2 = nc.declare_dram_parameter("p2", [128, EC], BF16, isOutput=False)
    p3 = nc.declare_dram_parameter("p3", [128, EC], BF16, isOutput=False)
    uT = nc.declare_dram_parameter("uT", [D, N_GRAPHS], F32, isOutput=False)
    idxw = nc.declare_dram_parameter("idxw", [128, EC // 16], I16, isOutput=False)
    # stacked bf16 weights
    wa = nc.declare_dram_parameter("wa", [128, D], BF16, isOutput=False)  # [Ws_hi;Wd_hi]
    wb = nc.declare_dram_parameter("wb", [128, D], BF16, isOutput=False)  # [Ws_lo;Wd_lo]
    wc = nc.declare_dram_parameter("wc", [128, D], BF16, isOutput=False)  # [We_hi;We_lo]
    wd = nc.declare_dram_parameter("wd", [D, D], BF16, isOutput=False)  # We_lo
    w2h = nc.declare_dram_parameter("w2h", [D, D], BF16, isOutput=False)
    w2l = nc.declare_dram_parameter("w2l", [D, D], BF16, isOutput=False)
    w1u = nc.declare_dram_parameter("w1u", [D, D], F32, isOutput=False)
    b1 = nc.declare_dram_parameter("b1", [D, 1], F32, isOutput=False)
    b2 = nc.declare_dram_parameter("b2", [D, 1], F32, isOutput=False)
    sid = nc.declare_dram_parameter("sid", [128, D], BF16, isOutput=False)
    outT = nc.declare_dram_parameter("outT", [D, EC], F32, isOutput=True)

    with tile.TileContext(nc) as tc:
        with (
            tc.tile_pool(name="const", bufs=1) as const_pool,
            tc.tile_pool(name="dram", bufs=1, space="DRAM") as dram_pool,
            tc.tile_pool(name="vprep", bufs=2) as vprep_pool,
            tc.tile_pool(name="dense", bufs=2) as dense_pool,
            tc.tile_pool(name="gath", bufs=4) as gath_pool,
            tc.tile_pool(name="outb", bufs=2) as out_pool,
            tc.tile_pool(name="hT", bufs=4) as h_pool,
            tc.tile_pool(name="psv", bufs=1, space="PSUM") as psv_pool,
            tc.tile_pool(name="psh", bufs=4, space="PSUM") as psh_pool,
            tc.tile_pool(name="pso", bufs=3, space="PSUM") as pso_pool,
        ):
            # ---- constants / one-time loads ----
            wa_sb = const_pool.tile([128, D], BF16, tag="wa")
            wb_sb = const_pool.tile([128, D], BF16, tag="wb")
            wc_sb = const_pool.tile([128, D], BF16, tag="wc")
            wd_sb = const_pool.tile([D, D], BF16, tag="wd")
            w2h_sb = const_pool.tile([D, D], BF16, tag="w2h")
            w2l_sb = const_pool.tile([D, D], BF16, tag="w2l")
            w1u_sb = const_pool.tile([D, D], F32, tag="w1u")
            b1_sb = const_pool.tile([D, 1], F32, tag="b1")
            b2_sb = const_pool.tile([D, 1], F32, tag="b2")
            sid_sb = const_pool.tile([128, D], BF16, tag="sid")
            uT_sb = const_pool.tile([D, N_GRAPHS], F32, tag="uT")
            idx_sb = const_pool.tile([128, EC // 16], I16, tag="idx")

            nc.sync.dma_start(wa_sb[:], wa[:])
            nc.sync.dma_start(wb_sb[:], wb[:])
            nc.sync.dma_start(wc_sb[:], wc[:])
            nc.sync.dma_start(wd_sb[:], wd[:])
            nc.sync.dma_start(w2h_sb[:], w2h[:])
            nc.sync.dma_start(w2l_sb[:], w2l[:])
            nc.sync.dma_start(w1u_sb[:], w1u[:])
            nc.sync.dma_start(b1_sb[:], b1[:])
            nc.sync.dma_start(b2_sb[:], b2[:])
            nc.sync.dma_start(sid_sb[:], sid[:])
            nc.sync.dma_start(uT_sb[:], uT[:])
            nc.sync.dma_start(idx_sb[:], idxw[:])

            # ---- one-time: v = u @ W1u, exact bf16 hi/lo split -> vtab DRAM ----
            vtab = dram_pool.tile([N_GRAPHS, 2 * D], BF16)
            for k in range(N_GRAPHS // 128):
                psv = psv_pool.tile([128, D], F32, tag="psv")
                nc.tensor.matmul(
                    psv[:],
                    uT_sb[:, k * 128 : (k + 1) * 128],
                    w1u_sb[:],
                    start=True,
                    stop=True,
                )
                vtab_sb = vprep_pool.tile([128, 2 * D], BF16, tag="vtab_sb")
                vhi_f32 = vprep_pool.tile([128, D], F32, tag="vhi_f32")
                # hi = bf16(v)
                nc.scalar.activation(
                    vtab_sb[:, 0:D], psv[:], mybir.ActivationFunctionType.Copy
                )
                nc.vector.tensor_copy(vhi_f32[:], vtab_sb[:, 0:D])
                # lo = bf16(v - hi)
                nc.vector.tensor_sub(vtab_sb[:, D : 2 * D], psv[:], vhi_f32[:])
                nc.sync.dma_start(vtab[k * 128 : (k + 1) * 128, :], vtab_sb[:])

            # ---- main loop over chunks of edges ----
            off = 0
            for csz in CHUNKS:
                gT = gath_pool.tile([128, 1, csz], BF16, tag="gT")
                nc.gpsimd.dma_gather(
                    gT[:],
                    vtab[:],
                    idx_sb[:, off // 16 : (off + csz) // 16],
                    num_idxs=csz,
                    num_idxs_reg=csz,
                    elem_size=2 * D,
                    transpose=True,
                    single_packet=False,
                )
                goff = 0

                t1c = dense_pool.tile([128, csz], BF16, tag="t1c")
                t2c = dense_pool.tile([128, csz], BF16, tag="t2c")
                t3c = dense_pool.tile([128, csz], BF16, tag="t3c")
                nc.sync.dma_start(t1c[:], p1[:, off : off + csz])
                nc.sync.dma_start(t2c[:], p2[:, off : off + csz])
                nc.sync.dma_start(t3c[:], p3[:, off : off + csz])

                oC = out_pool.tile([D, csz], F32, tag="oC")

                for t0 in range(0, csz, TS):
                    t1 = t0 + TS
                    psh = psh_pool.tile([D, TS], F32, tag="psh")
                    nc.tensor.matmul(
                        psh[:], wa_sb[:], t1c[:, t0:t1], start=True, stop=False
                    )
                    nc.tensor.matmul(
                        psh[:], wa_sb[:], t2c[:, t0:t1], start=False, stop=False
                    )
                    nc.tensor.matmul(
                        psh[:], wb_sb[:], t1c[:, t0:t1], start=False, stop=False
                    )
                    nc.tensor.matmul(
                        psh[:], wc_sb[:], t3c[:, t0:t1], start=False, stop=False
                    )
                    nc.tensor.matmul(
                        psh[:], wd_sb[:], t3c[0:D, t0:t1], start=False, stop=False
                    )
                    nc.tensor.matmul(
                        psh[:],
                        sid_sb[:],
                        gT[:, 0, goff + t0 : goff + t1],
                        start=False,
                        stop=True,
                    )
                    # h = relu(psh + b1); split into exact bf16 hi + lo
                    h_hi = h_pool.tile([D, TS], BF16, tag="h_hi")
                    nc.scalar.activation(
                        h_hi[:],
                        psh[:],
                        mybir.ActivationFunctionType.Relu,
                        bias=b1_sb[:],
                    )
                    h_f = h_pool.tile([D, TS], F32, tag="h_f")
                    nc.vector.tensor_scalar(
                        h_f[:],
                        psh[:],
                        b1_sb[:],
                        0.0,
                        op0=mybir.AluOpType.add,
                        op1=mybir.AluOpType.max,
                    )
                    h_lo = h_pool.tile([D, TS], BF16, tag="h_lo")
                    nc.vector.tensor_sub(h_lo[:], h_f[:], h_hi[:])

                    pso = pso_pool.tile([D, TS], F32, tag="pso")
                    nc.tensor.matmul(
                        pso[:], w2h_sb[:], h_hi[:], start=True, stop=False
                    )
                    nc.tensor.matmul(
                        pso[:], w2h_sb[:], h_lo[:], start=False, stop=False
                    )
                    nc.tensor.matmul(
                        pso[:], w2l_sb[:], h_hi[:], start=False, stop=True
                    )
                    nc.scalar.activation(
                        oC[:, t0:t1],
                        pso[:],
                        mybir.ActivationFunctionType.Relu,
                        bias=b2_sb[:],
                    )

                nc.sync.dma_start(outT[:, off : off + csz], oC[:])
                off += csz

    nc.compile()
    return nc


_NC = None


def get_nc():
    global _NC
    if _NC is None:
        _NC = build_nc()
    return _NC


def _hi_lo(x):
    """Exact bf16 hi/lo split of an fp32 array."""
    hi = x.astype(BF16_NP)
    lo = (x - hi.astype(np.float32)).astype(BF16_NP)
    return hi, lo


def prep_inputs(src, dest, edge_attr, u, batch, W1, b1, W2, b2):
    """Host-side shard + layout prep. Returns list of per-core input dicts."""
    E_pad = N_CORES * EC

    def shard_pad(x):
        xp = np.zeros((E_pad, D), dtype=np.float32)
        xp[:N_EDGES] = x
        return xp.reshape(N_CORES, EC, D)

    src_s = shard_pad(src)
    dest_s = shard_pad(dest)
    ea_s = shard_pad(edge_attr)

    bp = np.zeros((E_pad,), dtype=np.int16)
    bp[:N_EDGES] = batch.astype(np.int16)
    bp = bp.reshape(N_CORES, EC // 16, 16)
    idx_s = [np.ascontiguousarray(np.tile(bp[c].T, (8, 1))) for c in range(N_CORES)]

    uT = np.ascontiguousarray(u.T.astype(np.float32))
    W1 = np.asarray(W1, dtype=np.float32)
    W2 = np.asarray(W2, dtype=np.float32)
    ws_hi, ws_lo = _hi_lo(W1[0:64])
    wd_hi, wd_lo = _hi_lo(W1[64:128])
    we_hi, we_lo = _hi_lo(W1[128:192])
    w2_hi, w2_lo = _hi_lo(W2)

    sid = np.zeros((128, D), dtype=BF16_NP)
    for m in range(D):
        sid[m, m] = 1.0
        sid[m + D, m] = 1.0

    common = {
        "uT": uT,
        "wa": np.vstack([ws_hi, wd_hi]),
        "wb": np.vstack([ws_lo, wd_lo]),
        "wc": np.vstack([we_hi, we_hi]),
        "wd": np.ascontiguousarray(we_lo),
        "w2h": np.ascontiguousarray(w2_hi),
        "w2l": np.ascontiguousarray(w2_lo),
        "w1u": np.ascontiguousarray(W1[192:256]),
        "b1": np.asarray(b1, dtype=np.float32).reshape(D, 1),
        "b2": np.asarray(b2, dtype=np.float32).reshape(D, 1),
        "sid": sid,
    }
    in_maps = []
    for c in range(N_CORES):
        s_hi, s_lo = _hi_lo(src_s[c])
        d_hi, d_lo = _hi_lo(dest_s[c])
        e_hi, e_lo = _hi_lo(ea_s[c])
        m = dict(common)
        m["p1"] = np.ascontiguousarray(np.vstack([s_hi.T, d_hi.T]))
        m["p2"] = np.ascontiguousarray(np.vstack([s_lo.T, d_lo.T]))
        m["p3"] = np.ascontiguousarray(np.vstack([e_hi.T, e_lo.T]))
        m["idxw"] = idx_s[c]
        in_maps.append(m)
    return in_maps


def kernel(src, dest, edge_attr, u, batch, W1, b1, W2, b2):
    global LAST_RESULTS
    nc = get_nc()
    in_maps = prep_inputs(
        np.asarray(src, dtype=np.float32),
        np.asarray(dest, dtype=np.float32),
        np.asarray(edge_attr, dtype=np.float32),
        np.asarray(u, dtype=np.float32),
        np.asarray(batch),
        W1,
        b1,
        W2,
        b2,
    )
    res = run_bass_kernel_spmd(nc, in_maps, core_ids=list(range(N_CORES)), trace=TRACE)
    LAST_RESULTS = res
    out = np.empty((N_CORES * EC, D), dtype=np.float32)
    for c in range(N_CORES):
        out[c * EC : (c + 1) * EC] = res.results[c]["outT"].T
    return out[:N_EDGES]


# revision 29
# speedup vs baseline: 1.0103x; 1.0103x over previous
"""Trainium2 Bass kernel for the MEGNet EdgeModel MLP.

Computes out = relu(relu(concat([src, dest, edge_attr, u[batch]]) @ W1 + b1) @ W2 + b2)
for 500k edges, sharded data-parallel over edges across 8 NeuronCores.

Device-side layout is feature-major (features on SBUF partitions, edges on the
free dim): the PE contracts over partitions with small stationary weights and
512-edge column blocks as the moving operand.

All matmuls run in bf16 with exact hi/lo splitting (x = x_hi + x_lo, each bf16;
W likewise), keeping full fp32-level precision while streaming at bf16 rates
(measured 4.2x faster than fp32 matmuls on this part):

    x@W ~= x_hi@W_hi + x_lo@W_hi + x_hi@W_lo      (dropped x_lo@W_lo ~ 2^-18)

src/dest pairs are packed on 128 partitions so one K=128 matmul covers both.

The u-gather term uses a table trick: v = u @ W1[192:256] is computed once
on-device in fp32, split into exact bf16 hi/lo halves packed as [1024, 128]
bf16 rows in DRAM. dma_gather(transpose=True) gathers per-edge rows directly
into feature-major layout (partitions 0..63 = v_hi, 64..127 = v_lo) and a
stacked-identity bf16 matmul adds v_hi + v_lo into the same PSUM accumulation.

Host side only reshapes/transposes/splits for sharding and converts batch
indices to the int16 wrapped layout dma_gather expects.
"""

import sys

import numpy as np

sys.path.insert(0, "/opt/trn_rl_repo")

import ml_dtypes  # noqa: E402

import concourse.bacc as bacc  # noqa: E402
import concourse.tile as tile  # noqa: E402
from concourse import mybir  # noqa: E402
from concourse.bass_utils import run_bass_kernel_spmd  # noqa: E402

N_EDGES = 500_000
D = 64
N_GRAPHS = 1024
N_CORES = 8

TS = 512  # edge tile (matmul moving free dim)
EC = 63488  # per-core padded edge count: 15 x 4096 + 1 x 2048
CHUNKS = [4096] * 15 + [2048]
assert sum(CHUNKS) == EC

F32 = mybir.dt.float32
BF16 = mybir.dt.bfloat16
I16 = mybir.dt.int16
BF16_NP = ml_dtypes.bfloat16

TRACE = False
LAST_RESULTS = None


def build_nc():
    nc = bacc.Bacc("TRN2")

    # packed bf16 activations, feature-major:
    #   p1 = [srcT_hi ; destT_hi], p2 = [srcT_lo ; destT_lo], p3 = [eaT_hi ; eaT_lo]
    p1 = nc.declare_dram_parameter("p1", [128, EC], BF16, isOutput=False)
    p2 = nc.declare_dram_parameter("p2", [128, EC], BF16, isOutput=False)
    p3 = nc.declare_dram_parameter("p3", [128, EC], BF16, isOutput=False)
    uT = nc.declare_dram_parameter("uT", [D, N_GRAPHS], F32, isOutput=False)
    idxw = nc.declare_dram_parameter("idxw", [128, EC // 16], I16, isOutput=False)
    # stacked bf16 weights
    wa = nc.declare_dram_parameter("wa", [128, D], BF16, isOutput=False)  # [Ws_hi;Wd_hi]
    wb = nc.declare_dram_parameter("wb", [128, D], BF16, isOutput=False)  # [Ws_lo;Wd_lo]
    wc = nc.declare_dram_parameter("wc", [128, D], BF16, isOutput=False)  # [We_hi;We_lo]
    wd = nc.declare_dram_parameter("wd", [D, D], BF16, isOutput=False)  # We_lo
    w2h = nc.declare_dram_parameter("w2h", [D, D], BF16, isOutput=False)
    w2l = nc.declare_dram_parameter("w2l", [D, D], BF16, isOutput=False)
    w1u = nc.declare_dram_parameter("w1u", [D, D], F32, isOutput=False)
    b1 = nc.declare_dram_parameter("b1", [D, 1], F32, isOutput=False)
    b2 = nc.declare_dram_parameter("b2", [D, 1], F32, isOutput=False)
    sid = nc.declare_dram_parameter("sid", [128, D], BF16, isOutput=False)
    outT = nc.declare_dram_parameter("outT", [D, EC], F32, isOutput=True)

    with tile.TileContext(nc) as tc:
        with (
            tc.tile_pool(name="const", bufs=1) as const_pool,
            tc.tile_pool(name="dram", bufs=1, space="DRAM") as dram_pool,
            tc.tile_pool(name="vprep", bufs=2) as vprep_pool,
            tc.tile_pool(name="dense", bufs=2) as dense_pool,
            tc.tile_pool(name="gath", bufs=4) as gath_pool,
            tc.tile_pool(name="outb", bufs=2) as out_pool,
            tc.tile_pool(name="hT", bufs=4) as h_pool,
            tc.tile_pool(name="psv", bufs=2, space="PSUM") as psv_pool,
            tc.tile_pool(name="psh", bufs=4, space="PSUM") as psh_pool,
            tc.tile_pool(name="pso", bufs=2, space="PSUM") as pso_pool,
        ):
            # ---- constants / one-time loads ----
            wa_sb = const_pool.tile([128, D], BF16, tag="wa")
            wb_sb = const_pool.tile([128, D], BF16, tag="wb")
            wc_sb = const_pool.tile([128, D], BF16, tag="wc")
            wd_sb = const_pool.tile([D, D], BF16, tag="wd")
            w2h_sb = const_pool.tile([D, D], BF16, tag="w2h")
            w2l_sb = const_pool.tile([D, D], BF16, tag="w2l")
            w1u_sb = const_pool.tile([D, D], F32, tag="w1u")
            b1_sb = const_pool.tile([D, 1], F32, tag="b1")
            b2_sb = const_pool.tile([D, 1], F32, tag="b2")
            sid_sb = const_pool.tile([128, D], BF16, tag="sid")
            uT_sb = const_pool.tile([D, N_GRAPHS], F32, tag="uT")
            idx_sb = const_pool.tile([128, EC // 16], I16, tag="idx")

            nc.sync.dma_start(wa_sb[:], wa[:])
            nc.sync.dma_start(wb_sb[:], wb[:])
            nc.sync.dma_start(wc_sb[:], wc[:])
            nc.sync.dma_start(wd_sb[:], wd[:])
            nc.sync.dma_start(w2h_sb[:], w2h[:])
            nc.sync.dma_start(w2l_sb[:], w2l[:])
            nc.sync.dma_start(w1u_sb[:], w1u[:])
            nc.sync.dma_start(b1_sb[:], b1[:])
            nc.sync.dma_start(b2_sb[:], b2[:])
            nc.sync.dma_start(sid_sb[:], sid[:])
            nc.sync.dma_start(uT_sb[:], uT[:])
            nc.sync.dma_start(idx_sb[:], idxw[:])

            # ---- one-time: v = u @ W1u, exact bf16 hi/lo split -> vtab DRAM ----
            vtab = dram_pool.tile([N_GRAPHS, 2 * D], BF16)
            for k in range(N_GRAPHS // 128):
                psv = psv_pool.tile([128, D], F32, tag="psv")
                nc.tensor.matmul(
                    psv[:],
                    uT_sb[:, k * 128 : (k + 1) * 128],
                    w1u_sb[:],
                    start=True,
                    stop=True,
                )
                vtab_sb = vprep_pool.tile([128, 2 * D], BF16, tag="vtab_sb")
                vhi_f32 = vprep_pool.tile([128, D], F32, tag="vhi_f32")
                # hi = bf16(v)
                nc.scalar.activation(
                    vtab_sb[:, 0:D], psv[:], mybir.ActivationFunctionType.Copy
                )
                nc.vector.tensor_copy(vhi_f32[:], vtab_sb[:, 0:D])
                # lo = bf16(v - hi)
                nc.vector.tensor_sub(vtab_sb[:, D : 2 * D], psv[:], vhi_f32[:])
                nc.sync.dma_start(vtab[k * 128 : (k + 1) * 128, :], vtab_sb[:])

            # ---- main loop over chunks of edges ----
            off = 0
            for csz in CHUNKS:
                gT = gath_pool.tile([128, 1, csz], BF16, tag="gT")
                nc.gpsimd.dma_gather(
                    gT[:],
                    vtab[:],
                    idx_sb[:, off // 16 : (off + csz) // 16],
                    num_idxs=csz,
                    num_idxs_reg=csz,
                    elem_size=2 * D,
                    transpose=True,
                    single_packet=False,
                )
                goff = 0

                t1c = dense_pool.tile([128, csz], BF16, tag="t1c")
                t2c = dense_pool.tile([128, csz], BF16, tag="t2c")
                t3c = dense_pool.tile([128, csz], BF16, tag="t3c")
                nc.sync.dma_start(t1c[:], p1[:, off : off + csz])
                nc.sync.dma_start(t2c[:], p2[:, off : off + csz])
                nc.sync.dma_start(t3c[:], p3[:, off : off + csz])

                oC = out_pool.tile([D, csz], F32, tag="oC")

                for t0 in range(0, csz, TS):
                    t1 = t0 + TS
                    psh = psh_pool.tile([D, TS], F32, tag="psh")
                    nc.tensor.matmul(
                        psh[:], wa_sb[:], t1c[:, t0:t1], start=True, stop=False
                    )
                    nc.tensor.matmul(
                        psh[:], wa_sb[:], t2c[:, t0:t1], start=False, stop=False
                    )
                    nc.tensor.matmul(
                        psh[:], wb_sb[:], t1c[:, t0:t1], start=False, stop=False
                    )
                    nc.tensor.matmul(
                        psh[:], wc_sb[:], t3c[:, t0:t1], start=False, stop=False
                    )
                    nc.tensor.matmul(
                        psh[:], wd_sb[:], t3c[0:D, t0:t1], start=False, stop=False
                    )
                    nc.tensor.matmul(
                        psh[:],
                        sid_sb[:],
                        gT[:, 0, goff + t0 : goff + t1],
                        start=False,
                        stop=True,
                    )
                    # h = relu(psh + b1); split into exact bf16 hi + lo
                    h_hi = h_pool.tile([D, TS], BF16, tag="h_hi")
                    nc.scalar.activation(
                        h_hi[:],
                        psh[:],
                        mybir.ActivationFunctionType.Relu,
                        bias=b1_sb[:],
                    )
                    h_f = h_pool.tile([D, TS], F32, tag="h_f")
                    nc.vector.tensor_scalar(
                        h_f[:],
                        psh[:],
                        b1_sb[:],
                        0.0,
                        op0=mybir.AluOpType.add,
                        op1=mybir.AluOpType.max,
                    )
                    h_lo = h_pool.tile([D, TS], BF16, tag="h_lo")
                    nc.vector.tensor_sub(h_lo[:], h_f[:], h_hi[:])

                    pso = pso_pool.tile([D, TS], F32, tag="pso")
                    nc.tensor.matmul(
                        pso[:], w2h_sb[:], h_hi[:], start=True, stop=False
                    )
                    nc.tensor.matmul(
                        pso[:], w2h_sb[:], h_lo[:], start=False, stop=False
                    )
                    nc.tensor.matmul(
                        pso[:], w2l_sb[:], h_hi[:], start=False, stop=True
                    )
                    nc.scalar.activation(
                        oC[:, t0:t1],
                        pso[:],
                        mybir.ActivationFunctionType.Relu,
                        bias=b2_sb[:],
                    )

                nc.sync.dma_start(outT[:, off : off + csz], oC[:])
                off += csz

    nc.compile()
    return nc


_NC = None


def get_nc():
    global _NC
    if _NC is None:
        _NC = build_nc()
    return _NC


def _hi_lo(x):
    """Exact bf16 hi/lo split of an fp32 array."""
    hi = x.astype(BF16_NP)
    lo = (x - hi.astype(np.float32)).astype(BF16_NP)
    return hi, lo


def prep_inputs(src, dest, edge_attr, u, batch, W1, b1, W2, b2):
    """Host-side shard + layout prep. Returns list of per-core input dicts."""
    E_pad = N_CORES * EC

    def shard_pad(x):
        xp = np.zeros((E_pad, D), dtype=np.float32)
        xp[:N_EDGES] = x
        return xp.reshape(N_CORES, EC, D)

    src_s = shard_pad(src)
    dest_s = shard_pad(dest)
    ea_s = shard_pad(edge_attr)

    bp = np.zeros((E_pad,), dtype=np.int16)
    bp[:N_EDGES] = batch.astype(np.int16)
    bp = bp.reshape(N_CORES, EC // 16, 16)
    idx_s = [np.ascontiguousarray(np.tile(bp[c].T, (8, 1))) for c in range(N_CORES)]

    uT = np.ascontiguousarray(u.T.astype(np.float32))
    W1 = np.asarray(W1, dtype=np.float32)
    W2 = np.asarray(W2, dtype=np.float32)
    ws_hi, ws_lo = _hi_lo(W1[0:64])
    wd_hi, wd_lo = _hi_lo(W1[64:128])
    we_hi, we_lo = _hi_lo(W1[128:192])
    w2_hi, w2_lo = _hi_lo(W2)

    sid = np.zeros((128, D), dtype=BF16_NP)
    for m in range(D):
        sid[m, m] = 1.0
        sid[m + D, m] = 1.0

    common = {
        "uT": uT,
        "wa": np.vstack([ws_hi, wd_hi]),
        "wb": np.vstack([ws_lo, wd_lo]),
        "wc": np.vstack([we_hi, we_hi]),
        "wd": np.ascontiguousarray(we_lo),
        "w2h": np.ascontiguousarray(w2_hi),
        "w2l": np.ascontiguousarray(w2_lo),
        "w1u": np.ascontiguousarray(W1[192:256]),
        "b1": np.asarray(b1, dtype=np.float32).reshape(D, 1),
        "b2": np.asarray(b2, dtype=np.float32).reshape(D, 1),
        "sid": sid,
    }
    in_maps = []
    for c in range(N_CORES):
        s_hi, s_lo = _hi_lo(src_s[c])
        d_hi, d_lo = _hi_lo(dest_s[c])
        e_hi, e_lo = _hi_lo(ea_s[c])
        m = dict(common)
        m["p1"] = np.ascontiguousarray(np.vstack([s_hi.T, d_hi.T]))
        m["p2"] = np.ascontiguousarray(np.vstack([s_lo.T, d_lo.T]))
        m["p3"] = np.ascontiguousarray(np.vstack([e_hi.T, e_lo.T]))
        m["idxw"] = idx_s[c]
        in_maps.append(m)
    return in_maps


def kernel(src, dest, edge_attr, u, batch, W1, b1, W2, b2):
    global LAST_RESULTS
    nc = get_nc()
    in_maps = prep_inputs(
        np.asarray(src, dtype=np.float32),
        np.asarray(dest, dtype=np.float32),
        np.asarray(edge_attr, dtype=np.float32),
        np.asarray(u, dtype=np.float32),
        np.asarray(batch),
        W1,
        b1,
        W2,
        b2,
    )
    res = run_bass_kernel_spmd(nc, in_maps, core_ids=list(range(N_CORES)), trace=TRACE)
    LAST_RESULTS = res
    out = np.empty((N_CORES * EC, D), dtype=np.float32)
    for c in range(N_CORES):
        out[c * EC : (c + 1) * EC] = res.results[c]["outT"].T
    return out[:N_EDGES]


# revision 31
# speedup vs baseline: 1.0130x; 1.0026x over previous
"""Trainium2 Bass kernel for the MEGNet EdgeModel MLP.

Computes out = relu(relu(concat([src, dest, edge_attr, u[batch]]) @ W1 + b1) @ W2 + b2)
for 500k edges, sharded data-parallel over edges across 8 NeuronCores.

Device-side layout is feature-major (features on SBUF partitions, edges on the
free dim): the PE contracts over partitions with small stationary weights and
512-edge column blocks as the moving operand.

All matmuls run in bf16 with exact hi/lo splitting (x = x_hi + x_lo, each bf16;
W likewise), keeping full fp32-level precision while streaming at bf16 rates
(measured 4.2x faster than fp32 matmuls on this part):

    x@W ~= x_hi@W_hi + x_lo@W_hi + x_hi@W_lo      (dropped x_lo@W_lo ~ 2^-18)

src/dest pairs are packed on 128 partitions so one K=128 matmul covers both.

The u-gather term uses a table trick: v = u @ W1[192:256] is computed once
on-device in fp32, split into exact bf16 hi/lo halves packed as [1024, 128]
bf16 rows in DRAM. dma_gather(transpose=True) gathers per-edge rows directly
into feature-major layout (partitions 0..63 = v_hi, 64..127 = v_lo) and a
stacked-identity bf16 matmul adds v_hi + v_lo into the same PSUM accumulation.

Host side only reshapes/transposes/splits for sharding and converts batch
indices to the int16 wrapped layout dma_gather expects.
"""

import sys

import numpy as np

sys.path.insert(0, "/opt/trn_rl_repo")

import ml_dtypes  # noqa: E402

import concourse.bacc as bacc  # noqa: E402
import concourse.tile as tile  # noqa: E402
from concourse import mybir  # noqa: E402
from concourse.bass_utils import run_bass_kernel_spmd  # noqa: E402

N_EDGES = 500_000
D = 64
N_GRAPHS = 1024
N_CORES = 8

TS = 512  # edge tile (matmul moving free dim)
EC = 63488  # per-core padded edge count: 15 x 4096 + 1 x 2048
CHUNKS = [4096] * 15 + [2048]
assert sum(CHUNKS) == EC

F32 = mybir.dt.float32
BF16 = mybir.dt.bfloat16
I16 = mybir.dt.int16
BF16_NP = ml_dtypes.bfloat16

TRACE = False
LAST_RESULTS = None


def build_nc():
    nc = bacc.Bacc("TRN2")

    # packed bf16 activations, feature-major:
    #   p1 = [srcT_hi ; destT_hi], p2 = [srcT_lo ; destT_lo], p3 = [eaT_hi ; eaT_lo]
    p1 = nc.declare_dram_parameter("p1", [128, EC], BF16, isOutput=False)
    p2 = nc.declare_dram_parameter("p2", [128, EC], BF16, isOutput=False)
    p3 = nc.declare_dram_parameter("p3", [128, EC], BF16, isOutput=False)
    uT = nc.declare_dram_parameter("uT", [D, N_GRAPHS], F32, isOutput=False)
    idxw = nc.declare_dram_parameter("idxw", [128, EC // 16], I16, isOutput=False)
    # stacked bf16 weights
    wa = nc.declare_dram_parameter("wa", [128, D], BF16, isOutput=False)  # [Ws_hi;Wd_hi]
    wb = nc.declare_dram_parameter("wb", [128, D], BF16, isOutput=False)  # [Ws_lo;Wd_lo]
    wc = nc.declare_dram_parameter("wc", [128, D], BF16, isOutput=False)  # [We_hi;We_lo]
    wd = nc.declare_dram_parameter("wd", [D, D], BF16, isOutput=False)  # We_lo
    w2h = nc.declare_dram_parameter("w2h", [D, D], BF16, isOutput=False)
    w2l = nc.declare_dram_parameter("w2l", [D, D], BF16, isOutput=False)
    w1u = nc.declare_dram_parameter("w1u", [D, D], F32, isOutput=False)
    b1 = nc.declare_dram_parameter("b1", [D, 1], F32, isOutput=False)
    b2 = nc.declare_dram_parameter("b2", [D, 1], F32, isOutput=False)
    sid = nc.declare_dram_parameter("sid", [128, D], BF16, isOutput=False)
    outT = nc.declare_dram_parameter("outT", [D, EC], F32, isOutput=True)

    with tile.TileContext(nc) as tc:
        with (
            tc.tile_pool(name="const", bufs=1) as const_pool,
            tc.tile_pool(name="dram", bufs=1, space="DRAM") as dram_pool,
            tc.tile_pool(name="vprep", bufs=2) as vprep_pool,
            tc.tile_pool(name="dense", bufs=2) as dense_pool,
            tc.tile_pool(name="gath", bufs=3) as gath_pool,
            tc.tile_pool(name="outb", bufs=2) as out_pool,
            tc.tile_pool(name="hT", bufs=4) as h_pool,
            tc.tile_pool(name="psv", bufs=1, space="PSUM") as psv_pool,
            tc.tile_pool(name="psh", bufs=4, space="PSUM") as psh_pool,
            tc.tile_pool(name="pso", bufs=3, space="PSUM") as pso_pool,
        ):
            # ---- constants / one-time loads ----
            wa_sb = const_pool.tile([128, D], BF16, tag="wa")
            wb_sb = const_pool.tile([128, D], BF16, tag="wb")
            wc_sb = const_pool.tile([128, D], BF16, tag="wc")
            wd_sb = const_pool.tile([D, D], BF16, tag="wd")
            w2h_sb = const_pool.tile([D, D], BF16, tag="w2h")
            w2l_sb = const_pool.tile([D, D], BF16, tag="w2l")
            w1u_sb = const_pool.tile([D, D], F32, tag="w1u")
            b1_sb = const_pool.tile([D, 1], F32, tag="b1")
            b2_sb = const_pool.tile([D, 1], F32, tag="b2")
            sid_sb = const_pool.tile([128, D], BF16, tag="sid")
            uT_sb = const_pool.tile([D, N_GRAPHS], F32, tag="uT")
            idx_sb = const_pool.tile([128, EC // 16], I16, tag="idx")

            nc.sync.dma_start(wa_sb[:], wa[:])
            nc.sync.dma_start(wb_sb[:], wb[:])
            nc.sync.dma_start(wc_sb[:], wc[:])
            nc.sync.dma_start(wd_sb[:], wd[:])
            nc.sync.dma_start(w2h_sb[:], w2h[:])
            nc.sync.dma_start(w2l_sb[:], w2l[:])
            nc.sync.dma_start(w1u_sb[:], w1u[:])
            nc.sync.dma_start(b1_sb[:], b1[:])
            nc.sync.dma_start(b2_sb[:], b2[:])
            nc.sync.dma_start(sid_sb[:], sid[:])
            nc.sync.dma_start(uT_sb[:], uT[:])
            nc.sync.dma_start(idx_sb[:], idxw[:])

            # ---- one-time: v = u @ W1u, exact bf16 hi/lo split -> vtab DRAM ----
            vtab = dram_pool.tile([N_GRAPHS, 2 * D], BF16)
            for k in range(N_GRAPHS // 128):
                psv = psv_pool.tile([128, D], F32, tag="psv")
                nc.tensor.matmul(
                    psv[:],
                    uT_sb[:, k * 128 : (k + 1) * 128],
                    w1u_sb[:],
                    start=True,
                    stop=True,
                )
                vtab_sb = vprep_pool.tile([128, 2 * D], BF16, tag="vtab_sb")
                vhi_f32 = vprep_pool.tile([128, D], F32, tag="vhi_f32")
                # hi = bf16(v)
                nc.scalar.activation(
                    vtab_sb[:, 0:D], psv[:], mybir.ActivationFunctionType.Copy
                )
                nc.vector.tensor_copy(vhi_f32[:], vtab_sb[:, 0:D])
                # lo = bf16(v - hi)
                nc.vector.tensor_sub(vtab_sb[:, D : 2 * D], psv[:], vhi_f32[:])
                nc.sync.dma_start(vtab[k * 128 : (k + 1) * 128, :], vtab_sb[:])

            # ---- main loop over chunks of edges ----
            off = 0
            for csz in CHUNKS:
                gT = gath_pool.tile([128, 1, csz], BF16, tag="gT")
                nc.gpsimd.dma_gather(
                    gT[:],
                    vtab[:],
                    idx_sb[:, off // 16 : (off + csz) // 16],
                    num_idxs=csz,
                    num_idxs_reg=csz,
                    elem_size=2 * D,
                    transpose=True,
                    single_packet=False,
                )
                goff = 0

                t1c = dense_pool.tile([128, csz], BF16, tag="t1c")
                t2c = dense_pool.tile([128, csz], BF16, tag="t2c")
                t3c = dense_pool.tile([128, csz], BF16, tag="t3c")
                nc.sync.dma_start(t1c[:], p1[:, off : off + csz])
                nc.sync.dma_start(t2c[:], p2[:, off : off + csz])
                nc.sync.dma_start(t3c[:], p3[:, off : off + csz])

                oC = out_pool.tile([D, csz], F32, tag="oC")

                for t0 in range(0, csz, TS):
                    t1 = t0 + TS
                    psh = psh_pool.tile([D, TS], F32, tag="psh")
                    nc.tensor.matmul(
                        psh[:], wa_sb[:], t1c[:, t0:t1], start=True, stop=False
                    )
                    nc.tensor.matmul(
                        psh[:], wa_sb[:], t2c[:, t0:t1], start=False, stop=False
                    )
                    nc.tensor.matmul(
                        psh[:], wb_sb[:], t1c[:, t0:t1], start=False, stop=False
                    )
                    nc.tensor.matmul(
                        psh[:], wc_sb[:], t3c[:, t0:t1], start=False, stop=False
                    )
                    nc.tensor.matmul(
                        psh[:], wd_sb[:], t3c[0:D, t0:t1], start=False, stop=False
                    )
                    nc.tensor.matmul(
                        psh[:],
                        sid_sb[:],
                        gT[:, 0, goff + t0 : goff + t1],
                        start=False,
                        stop=True,
                    )
                    # h = relu(psh + b1); split into exact bf16 hi + lo
                    h_hi = h_pool.tile([D, TS], BF16, tag="h_hi")
                    nc.scalar.activation(
                        h_hi[:],
                        psh[:],
                        mybir.ActivationFunctionType.Relu,
                        bias=b1_sb[:],
                    )
                    h_f = h_pool.tile([D, TS], F32, tag="h_f")
                    nc.vector.tensor_scalar(
                        h_f[:],
                        psh[:],
                        b1_sb[:],
                        0.0,
                        op0=mybir.AluOpType.add,
                        op1=mybir.AluOpType.max,
                    )
                    h_lo = h_pool.tile([D, TS], BF16, tag="h_lo")
                    nc.vector.tensor_sub(h_lo[:], h_f[:], h_hi[:])

                    pso = pso_pool.tile([D, TS], F32, tag="pso")
                    nc.tensor.matmul(
                        pso[:], w2h_sb[:], h_hi[:], start=True, stop=False
                    )
                    nc.tensor.matmul(
                        pso[:], w2h_sb[:], h_lo[:], start=False, stop=False
                    )
                    nc.tensor.matmul(
                        pso[:], w2l_sb[:], h_hi[:], start=False, stop=True
                    )
                    nc.scalar.activation(
                        oC[:, t0:t1],
                        pso[:],
                        mybir.ActivationFunctionType.Relu,
                        bias=b2_sb[:],
                    )

                nc.sync.dma_start(outT[:, off : off + csz], oC[:])
                off += csz

    nc.compile()
    return nc


_NC = None


def get_nc():
    global _NC
    if _NC is None:
        _NC = build_nc()
    return _NC


def _hi_lo(x):
    """Exact bf16 hi/lo split of an fp32 array."""
    hi = x.astype(BF16_NP)
    lo = (x - hi.astype(np.float32)).astype(BF16_NP)
    return hi, lo


def prep_inputs(src, dest, edge_attr, u, batch, W1, b1, W2, b2):
    """Host-side shard + layout prep. Returns list of per-core input dicts."""
    E_pad = N_CORES * EC

    def shard_pad(x):
        xp = np.zeros((E_pad, D), dtype=np.float32)
        xp[:N_EDGES] = x
        return xp.reshape(N_CORES, EC, D)

    src_s = shard_pad(src)
    dest_s = shard_pad(dest)
    ea_s = shard_pad(edge_attr)

    bp = np.zeros((E_pad,), dtype=np.int16)
    bp[:N_EDGES] = batch.astype(np.int16)
    bp = bp.reshape(N_CORES, EC // 16, 16)
    idx_s = [np.ascontiguousarray(np.tile(bp[c].T, (8, 1))) for c in range(N_CORES)]

    uT = np.ascontiguousarray(u.T.astype(np.float32))
    W1 = np.asarray(W1, dtype=np.float32)
    W2 = np.asarray(W2, dtype=np.float32)
    ws_hi, ws_lo = _hi_lo(W1[0:64])
    wd_hi, wd_lo = _hi_lo(W1[64:128])
    we_hi, we_lo = _hi_lo(W1[128:192])
    w2_hi, w2_lo = _hi_lo(W2)

    sid = np.zeros((128, D), dtype=BF16_NP)
    for m in range(D):
        sid[m, m] = 1.0
        sid[m + D, m] = 1.0

    common = {
        "uT": uT,
        "wa": np.vstack([ws_hi, wd_hi]),
        "wb": np.vstack([ws_lo, wd_lo]),
        "wc": np.vstack([we_hi, we_hi]),
        "wd": np.ascontiguousarray(we_lo),
        "w2h": np.ascontiguousarray(w2_hi),
        "w2l": np.ascontiguousarray(w2_lo),
        "w1u": np.ascontiguousarray(W1[192:256]),
        "b1": np.asarray(b1, dtype=np.float32).reshape(D, 1),
        "b2": np.asarray(b2, dtype=np.float32).reshape(D, 1),
        "sid": sid,
    }
    in_maps = []
    for c in range(N_CORES):
        s_hi, s_lo = _hi_lo(src_s[c])
        d_hi, d_lo = _hi_lo(dest_s[c])
        e_hi, e_lo = _hi_lo(ea_s[c])
        m = dict(common)
        m["p1"] = np.ascontiguousarray(np.vstack([s_hi.T, d_hi.T]))
        m["p2"] = np.ascontiguousarray(np.vstack([s_lo.T, d_lo.T]))
        m["p3"] = np.ascontiguousarray(np.vstack([e_hi.T, e_lo.T]))
        m["idxw"] = idx_s[c]
        in_maps.append(m)
    return in_maps


def kernel(src, dest, edge_attr, u, batch, W1, b1, W2, b2):
    global LAST_RESULTS
    nc = get_nc()
    in_maps = prep_inputs(
        np.asarray(src, dtype=np.float32),
        np.asarray(dest, dtype=np.float32),
        np.asarray(edge_attr, dtype=np.float32),
        np.asarray(u, dtype=np.float32),
        np.asarray(batch),
        W1,
        b1,
        W2,
        b2,
    )
    res = run_bass_kernel_spmd(nc, in_maps, core_ids=list(range(N_CORES)), trace=TRACE)
    LAST_RESULTS = res
    out = np.empty((N_CORES * EC, D), dtype=np.float32)
    for c in range(N_CORES):
        out[c * EC : (c + 1) * EC] = res.results[c]["outT"].T
    return out[:N_EDGES]


# revision 33
# speedup vs baseline: 1.0234x; 1.0103x over previous
"""Trainium2 Bass kernel for the MEGNet EdgeModel MLP.

Computes out = relu(relu(concat([src, dest, edge_attr, u[batch]]) @ W1 + b1) @ W2 + b2)
for 500k edges, sharded data-parallel over edges across 8 NeuronCores.

Device-side layout is feature-major (features on SBUF partitions, edges on the
free dim): the PE contracts over partitions with small stationary weights and
512-edge column blocks as the moving operand.

All matmuls run in bf16 with exact hi/lo splitting (x = x_hi + x_lo, each bf16;
W likewise), keeping full fp32-level precision while streaming at bf16 rates
(measured 4.2x faster than fp32 matmuls on this part):

    x@W ~= x_hi@W_hi + x_lo@W_hi + x_hi@W_lo      (dropped x_lo@W_lo ~ 2^-18)

src/dest pairs are packed on 128 partitions so one K=128 matmul covers both.

The u-gather term uses a table trick: v = u @ W1[192:256] is computed once
on-device in fp32, split into exact bf16 hi/lo halves packed as [1024, 128]
bf16 rows in DRAM. dma_gather(transpose=True) gathers per-edge rows directly
into feature-major layout (partitions 0..63 = v_hi, 64..127 = v_lo) and a
stacked-identity bf16 matmul adds v_hi + v_lo into the same PSUM accumulation.

Host side only reshapes/transposes/splits for sharding and converts batch
indices to the int16 wrapped layout dma_gather expects.
"""

import sys

import numpy as np

sys.path.insert(0, "/opt/trn_rl_repo")

import ml_dtypes  # noqa: E402

import concourse.bacc as bacc  # noqa: E402
import concourse.tile as tile  # noqa: E402
from concourse import mybir  # noqa: E402
from concourse.bass_utils import run_bass_kernel_spmd  # noqa: E402

N_EDGES = 500_000
D = 64
N_GRAPHS = 1024
N_CORES = 8

TS = 512  # edge tile (matmul moving free dim)
EC = 63488  # per-core padded edge count: 15 x 4096 + 1 x 2048
CHUNKS = [4096] * 15 + [2048]
assert sum(CHUNKS) == EC

F32 = mybir.dt.float32
BF16 = mybir.dt.bfloat16
I16 = mybir.dt.int16
BF16_NP = ml_dtypes.bfloat16

TRACE = False
LAST_RESULTS = None


def build_nc():
    nc = bacc.Bacc("TRN2")

    # packed bf16 activations, feature-major:
    #   p1 = [srcT_hi ; destT_hi], p2 = [srcT_lo ; destT_lo], p3 = [eaT_hi ; eaT_lo]
    p1 = nc.declare_dram_parameter("p1", [128, EC], BF16, isOutput=False)
    p2 = nc.declare_dram_parameter("p2", [128, EC], BF16, isOutput=False)
    p3 = nc.declare_dram_parameter("p3", [128, EC], BF16, isOutput=False)
    uT = nc.declare_dram_parameter("uT", [D, N_GRAPHS], F32, isOutput=False)
    idxw = nc.declare_dram_parameter("idxw", [128, EC // 16], I16, isOutput=False)
    # stacked bf16 weights
    wa = nc.declare_dram_parameter("wa", [128, D], BF16, isOutput=False)  # [Ws_hi;Wd_hi]
    wb = nc.declare_dram_parameter("wb", [128, D], BF16, isOutput=False)  # [Ws_lo;Wd_lo]
    wc = nc.declare_dram_parameter("wc", [128, D], BF16, isOutput=False)  # [We_hi;We_lo]
    wd = nc.declare_dram_parameter("wd", [D, D], BF16, isOutput=False)  # We_lo
    w2h = nc.declare_dram_parameter("w2h", [D, D], BF16, isOutput=False)
    w2l = nc.declare_dram_parameter("w2l", [D, D], BF16, isOutput=False)
    w1u = nc.declare_dram_parameter("w1u", [D, D], F32, isOutput=False)
    b1 = nc.declare_dram_parameter("b1", [D, 1], F32, isOutput=False)
    b2 = nc.declare_dram_parameter("b2", [D, 1], F32, isOutput=False)
    sid = nc.declare_dram_parameter("sid", [128, D], BF16, isOutput=False)
    outT = nc.declare_dram_parameter("outT", [D, EC], F32, isOutput=True)

    with tile.TileContext(nc) as tc:
        with (
            tc.tile_pool(name="const", bufs=1) as const_pool,
            tc.tile_pool(name="dram", bufs=1, space="DRAM") as dram_pool,
            tc.tile_pool(name="vprep", bufs=2) as vprep_pool,
            tc.tile_pool(name="dense", bufs=2) as dense_pool,
            tc.tile_pool(name="gath", bufs=3) as gath_pool,
            tc.tile_pool(name="outb", bufs=2) as out_pool,
            tc.tile_pool(name="hT", bufs=4) as h_pool,
            tc.tile_pool(name="psv", bufs=1, space="PSUM") as psv_pool,
            tc.tile_pool(name="psh", bufs=4, space="PSUM") as psh_pool,
            tc.tile_pool(name="pso", bufs=3, space="PSUM") as pso_pool,
        ):
            # ---- constants / one-time loads ----
            wa_sb = const_pool.tile([128, D], BF16, tag="wa")
            wb_sb = const_pool.tile([128, D], BF16, tag="wb")
            wc_sb = const_pool.tile([128, D], BF16, tag="wc")
            wd_sb = const_pool.tile([D, D], BF16, tag="wd")
            w2h_sb = const_pool.tile([D, D], BF16, tag="w2h")
            w2l_sb = const_pool.tile([D, D], BF16, tag="w2l")
            w1u_sb = const_pool.tile([D, D], F32, tag="w1u")
            b1_sb = const_pool.tile([D, 1], F32, tag="b1")
            b2_sb = const_pool.tile([D, 1], F32, tag="b2")
            sid_sb = const_pool.tile([128, D], BF16, tag="sid")
            uT_sb = const_pool.tile([D, N_GRAPHS], F32, tag="uT")
            idx_sb = const_pool.tile([128, EC // 16], I16, tag="idx")

            nc.sync.dma_start(wa_sb[:], wa[:])
            nc.sync.dma_start(wb_sb[:], wb[:])
            nc.sync.dma_start(wc_sb[:], wc[:])
            nc.sync.dma_start(wd_sb[:], wd[:])
            nc.sync.dma_start(w2h_sb[:], w2h[:])
            nc.sync.dma_start(w2l_sb[:], w2l[:])
            nc.sync.dma_start(w1u_sb[:], w1u[:])
            nc.sync.dma_start(b1_sb[:], b1[:])
            nc.sync.dma_start(b2_sb[:], b2[:])
            nc.sync.dma_start(sid_sb[:], sid[:])
            nc.sync.dma_start(uT_sb[:], uT[:])
            nc.sync.dma_start(idx_sb[:], idxw[:])

            # ---- one-time: v = u @ W1u, exact bf16 hi/lo split -> vtab DRAM ----
            # all 8 graph-chunks land in ONE psum bank ([128, 8*64] fp32 = 2KB)
            # so the whole table is produced by 8 MMs + 1 ACT + 2 DVE + 2 DMAs
            vtab = dram_pool.tile([N_GRAPHS, 2 * D], BF16)
            psv = psv_pool.tile([128, N_GRAPHS // 128 * D], F32, tag="psv")
            for k in range(N_GRAPHS // 128):
                nc.tensor.matmul(
                    psv[:, k * D : (k + 1) * D],
                    uT_sb[:, k * 128 : (k + 1) * 128],
                    w1u_sb[:],
                    start=True,
                    stop=True,
                )
            vhi_sb = vprep_pool.tile([128, N_GRAPHS // 128, D], BF16, tag="vhi_sb")
            vhi_f32 = vprep_pool.tile([128, N_GRAPHS // 128 * D], F32, tag="vhi_f32")
            vlo_sb = vprep_pool.tile([128, N_GRAPHS // 128, D], BF16, tag="vlo_sb")
            # hi = bf16(v)
            nc.scalar.activation(
                vhi_sb.rearrange("p a b -> p (a b)"),
                psv[:],
                mybir.ActivationFunctionType.Copy,
            )
            nc.vector.tensor_copy(vhi_f32[:], vhi_sb.rearrange("p a b -> p (a b)"))
            # lo = bf16(v - hi)
            nc.vector.tensor_sub(
                vlo_sb.rearrange("p a b -> p (a b)"), psv[:], vhi_f32[:]
            )
            # vtab[128k + p, 0:64] = hi chunk k, [64:128] = lo chunk k
            vtab_rows = vtab[:].rearrange("(a p) c -> p a c", p=128)
            nc.sync.dma_start(vtab_rows[:, :, 0:D], vhi_sb[:])
            nc.sync.dma_start(vtab_rows[:, :, D : 2 * D], vlo_sb[:])

            # ---- main loop over chunks of edges ----
            off = 0
            for csz in CHUNKS:
                gT = gath_pool.tile([128, 1, csz], BF16, tag="gT")
                nc.gpsimd.dma_gather(
                    gT[:],
                    vtab[:],
                    idx_sb[:, off // 16 : (off + csz) // 16],
                    num_idxs=csz,
                    num_idxs_reg=csz,
                    elem_size=2 * D,
                    transpose=True,
                    single_packet=False,
                )
                goff = 0

                t1c = dense_pool.tile([128, csz], BF16, tag="t1c")
                t2c = dense_pool.tile([128, csz], BF16, tag="t2c")
                t3c = dense_pool.tile([128, csz], BF16, tag="t3c")
                nc.sync.dma_start(t1c[:], p1[:, off : off + csz])
                nc.sync.dma_start(t2c[:], p2[:, off : off + csz])
                nc.sync.dma_start(t3c[:], p3[:, off : off + csz])

                oC = out_pool.tile([D, csz], F32, tag="oC")

                for t0 in range(0, csz, TS):
                    t1 = t0 + TS
                    psh = psh_pool.tile([D, TS], F32, tag="psh")
                    nc.tensor.matmul(
                        psh[:], wa_sb[:], t1c[:, t0:t1], start=True, stop=False
                    )
                    nc.tensor.matmul(
                        psh[:], wa_sb[:], t2c[:, t0:t1], start=False, stop=False
                    )
                    nc.tensor.matmul(
                        psh[:], wb_sb[:], t1c[:, t0:t1], start=False, stop=False
                    )
                    nc.tensor.matmul(
                        psh[:], wc_sb[:], t3c[:, t0:t1], start=False, stop=False
                    )
                    nc.tensor.matmul(
                        psh[:], wd_sb[:], t3c[0:D, t0:t1], start=False, stop=False
                    )
                    nc.tensor.matmul(
                        psh[:],
                        sid_sb[:],
                        gT[:, 0, goff + t0 : goff + t1],
                        start=False,
                        stop=True,
                    )
                    # h = relu(psh + b1); split into exact bf16 hi + lo
                    h_hi = h_pool.tile([D, TS], BF16, tag="h_hi")
                    nc.scalar.activation(
                        h_hi[:],
                        psh[:],
                        mybir.ActivationFunctionType.Relu,
                        bias=b1_sb[:],
                    )
                    h_f = h_pool.tile([D, TS], F32, tag="h_f")
                    nc.vector.tensor_scalar(
                        h_f[:],
                        psh[:],
                        b1_sb[:],
                        0.0,
                        op0=mybir.AluOpType.add,
                        op1=mybir.AluOpType.max,
                    )
                    h_lo = h_pool.tile([D, TS], BF16, tag="h_lo")
                    nc.vector.tensor_sub(h_lo[:], h_f[:], h_hi[:])

                    pso = pso_pool.tile([D, TS], F32, tag="pso")
                    nc.tensor.matmul(
                        pso[:], w2h_sb[:], h_hi[:], start=True, stop=False
                    )
                    nc.tensor.matmul(
                        pso[:], w2h_sb[:], h_lo[:], start=False, stop=False
                    )
                    nc.tensor.matmul(
                        pso[:], w2l_sb[:], h_hi[:], start=False, stop=True
                    )
                    nc.scalar.activation(
                        oC[:, t0:t1],
                        pso[:],
                        mybir.ActivationFunctionType.Relu,
                        bias=b2_sb[:],
                    )

                nc.sync.dma_start(outT[:, off : off + csz], oC[:])
                off += csz

    nc.compile()
    return nc


_NC = None


def get_nc():
    global _NC
    if _NC is None:
        _NC = build_nc()
    return _NC


def _hi_lo(x):
    """Exact bf16 hi/lo split of an fp32 array."""
    hi = x.astype(BF16_NP)
    lo = (x - hi.astype(np.float32)).astype(BF16_NP)
    return hi, lo


def prep_inputs(src, dest, edge_attr, u, batch, W1, b1, W2, b2):
    """Host-side shard + layout prep. Returns list of per-core input dicts."""
    E_pad = N_CORES * EC

    def shard_pad(x):
        xp = np.zeros((E_pad, D), dtype=np.float32)
        xp[:N_EDGES] = x
        return xp.reshape(N_CORES, EC, D)

    src_s = shard_pad(src)
    dest_s = shard_pad(dest)
    ea_s = shard_pad(edge_attr)

    bp = np.zeros((E_pad,), dtype=np.int16)
    bp[:N_EDGES] = batch.astype(np.int16)
    bp = bp.reshape(N_CORES, EC // 16, 16)
    idx_s = [np.ascontiguousarray(np.tile(bp[c].T, (8, 1))) for c in range(N_CORES)]

    uT = np.ascontiguousarray(u.T.astype(np.float32))
    W1 = np.asarray(W1, dtype=np.float32)
    W2 = np.asarray(W2, dtype=np.float32)
    ws_hi, ws_lo = _hi_lo(W1[0:64])
    wd_hi, wd_lo = _hi_lo(W1[64:128])
    we_hi, we_lo = _hi_lo(W1[128:192])
    w2_hi, w2_lo = _hi_lo(W2)

    sid = np.zeros((128, D), dtype=BF16_NP)
    for m in range(D):
        sid[m, m] = 1.0
        sid[m + D, m] = 1.0

    common = {
        "uT": uT,
        "wa": np.vstack([ws_hi, wd_hi]),
        "wb": np.vstack([ws_lo, wd_lo]),
        "wc": np.vstack([we_hi, we_hi]),
        "wd": np.ascontiguousarray(we_lo),
        "w2h": np.ascontiguousarray(w2_hi),
        "w2l": np.ascontiguousarray(w2_lo),
        "w1u": np.ascontiguousarray(W1[192:256]),
        "b1": np.asarray(b1, dtype=np.float32).reshape(D, 1),
        "b2": np.asarray(b2, dtype=np.float32).reshape(D, 1),
        "sid": sid,
    }
    in_maps = []
    for c in range(N_CORES):
        s_hi, s_lo = _hi_lo(src_s[c])
        d_hi, d_lo = _hi_lo(dest_s[c])
        e_hi, e_lo = _hi_lo(ea_s[c])
        m = dict(common)
        m["p1"] = np.ascontiguousarray(np.vstack([s_hi.T, d_hi.T]))
        m["p2"] = np.ascontiguousarray(np.vstack([s_lo.T, d_lo.T]))
        m["p3"] = np.ascontiguousarray(np.vstack([e_hi.T, e_lo.T]))
        m["idxw"] = idx_s[c]
        in_maps.append(m)
    return in_maps


def kernel(src, dest, edge_attr, u, batch, W1, b1, W2, b2):
    global LAST_RESULTS
    nc = get_nc()
    in_maps = prep_inputs(
        np.asarray(src, dtype=np.float32),
        np.asarray(dest, dtype=np.float32),
        np.asarray(edge_attr, dtype=np.float32),
        np.asarray(u, dtype=np.float32),
        np.asarray(batch),
        W1,
        b1,
        W2,
        b2,
    )
    res = run_bass_kernel_spmd(nc, in_maps, core_ids=list(range(N_CORES)), trace=TRACE)
    LAST_RESULTS = res
    out = np.empty((N_CORES * EC, D), dtype=np.float32)
    for c in range(N_CORES):
        out[c * EC : (c + 1) * EC] = res.results[c]["outT"].T
    return out[:N_EDGES]


# revision 36
# speedup vs baseline: 1.3255x; 1.2952x over previous
"""Trainium2 Bass kernel for the MEGNet EdgeModel MLP.

Computes out = relu(relu(concat([src, dest, edge_attr, u[batch]]) @ W1 + b1) @ W2 + b2)
for 500k edges, sharded data-parallel over edges across 8 NeuronCores.

Device-side layout is feature-major (features on SBUF partitions, edges on the
free dim): the PE contracts over partitions with small stationary weights and
512-edge column blocks as the moving operand.

All matmuls run in bf16 with exact hi/lo splitting (x = x_hi + x_lo, each bf16;
W likewise), keeping full fp32-level precision while streaming at bf16 rates
(measured 4.2x faster than fp32 matmuls on this part):

    x@W ~= x_hi@W_hi + x_lo@W_hi + x_hi@W_lo      (dropped x_lo@W_lo ~ 2^-18)

src/dest pairs are packed on 128 partitions so one K=128 matmul covers both.

The u-gather term uses a table trick: v = u @ W1[192:256] is computed once
on-device in fp32, split into exact bf16 hi/lo halves packed as [1024, 128]
bf16 rows in DRAM. dma_gather(transpose=True) gathers per-edge rows directly
into feature-major layout (partitions 0..63 = v_hi, 64..127 = v_lo) and a
stacked-identity bf16 matmul adds v_hi + v_lo into the same PSUM accumulation.

Host side only reshapes/transposes/splits for sharding and converts batch
indices to the int16 wrapped layout dma_gather expects.
"""

import sys

import numpy as np

sys.path.insert(0, "/opt/trn_rl_repo")

import ml_dtypes  # noqa: E402

import concourse.bacc as bacc  # noqa: E402
import concourse.tile as tile  # noqa: E402
from concourse import mybir  # noqa: E402
from concourse.bass_utils import run_bass_kernel_spmd  # noqa: E402

N_EDGES = 500_000
D = 64
N_GRAPHS = 1024
N_CORES = 8

TS = 512  # edge tile (matmul moving free dim)
EC = 63488  # per-core padded edge count: 15 x 4096 + 1 x 2048
CHUNKS = [4096] * 15 + [2048]
assert sum(CHUNKS) == EC

F32 = mybir.dt.float32
BF16 = mybir.dt.bfloat16
I16 = mybir.dt.int16
BF16_NP = ml_dtypes.bfloat16

TRACE = False
LAST_RESULTS = None


def build_nc():
    nc = bacc.Bacc("TRN2")

    # packed bf16 activations, feature-major:
    #   p1 = [srcT_hi ; destT_hi], p2 = [srcT_lo ; destT_lo], p3 = [eaT_hi ; eaT_lo]
    p1 = nc.declare_dram_parameter("p1", [128, EC], BF16, isOutput=False)
    p2 = nc.declare_dram_parameter("p2", [128, EC], BF16, isOutput=False)
    p3 = nc.declare_dram_parameter("p3", [128, EC], BF16, isOutput=False)
    uT = nc.declare_dram_parameter("uT", [D, N_GRAPHS], F32, isOutput=False)
    idxw = nc.declare_dram_parameter("idxw", [128, EC // 32], I16, isOutput=False)
    # stacked bf16 weights
    wa = nc.declare_dram_parameter("wa", [128, D], BF16, isOutput=False)  # [Ws_hi;Wd_hi]
    wb = nc.declare_dram_parameter("wb", [128, D], BF16, isOutput=False)  # [Ws_lo;Wd_lo]
    wc = nc.declare_dram_parameter("wc", [128, D], BF16, isOutput=False)  # [We_hi;We_lo]
    wd = nc.declare_dram_parameter("wd", [D, D], BF16, isOutput=False)  # We_lo
    w2h = nc.declare_dram_parameter("w2h", [D, D], BF16, isOutput=False)
    w2l = nc.declare_dram_parameter("w2l", [D, D], BF16, isOutput=False)
    w1u = nc.declare_dram_parameter("w1u", [D, D], F32, isOutput=False)
    b1 = nc.declare_dram_parameter("b1", [D, 1], F32, isOutput=False)
    b2 = nc.declare_dram_parameter("b2", [D, 1], F32, isOutput=False)
    sid = nc.declare_dram_parameter("sid", [128, D], BF16, isOutput=False)
    outT = nc.declare_dram_parameter("outT", [D, EC], F32, isOutput=True)

    with tile.TileContext(nc) as tc:
        with (
            tc.tile_pool(name="const", bufs=1) as const_pool,
            tc.tile_pool(name="dram", bufs=1, space="DRAM") as dram_pool,
            tc.tile_pool(name="vprep", bufs=2) as vprep_pool,
            tc.tile_pool(name="dense", bufs=2) as dense_pool,
            tc.tile_pool(name="gath", bufs=3) as gath_pool,
            tc.tile_pool(name="outb", bufs=2) as out_pool,
            tc.tile_pool(name="hT", bufs=4) as h_pool,
            tc.tile_pool(name="psv", bufs=1, space="PSUM") as psv_pool,
            tc.tile_pool(name="psh", bufs=4, space="PSUM") as psh_pool,
            tc.tile_pool(name="pso", bufs=3, space="PSUM") as pso_pool,
        ):
            # ---- constants / one-time loads ----
            wa_sb = const_pool.tile([128, D], BF16, tag="wa")
            wb_sb = const_pool.tile([128, D], BF16, tag="wb")
            wc_sb = const_pool.tile([128, D], BF16, tag="wc")
            wd_sb = const_pool.tile([D, D], BF16, tag="wd")
            w2h_sb = const_pool.tile([D, D], BF16, tag="w2h")
            w2l_sb = const_pool.tile([D, D], BF16, tag="w2l")
            w1u_sb = const_pool.tile([D, D], F32, tag="w1u")
            b1_sb = const_pool.tile([D, 1], F32, tag="b1")
            b2_sb = const_pool.tile([D, 1], F32, tag="b2")
            sid_sb = const_pool.tile([128, D], BF16, tag="sid")
            uT_sb = const_pool.tile([D, N_GRAPHS], F32, tag="uT")
            idx_sb = const_pool.tile([128, EC // 32], I16, tag="idx")

            nc.sync.dma_start(wa_sb[:], wa[:])
            nc.sync.dma_start(wb_sb[:], wb[:])
            nc.sync.dma_start(wc_sb[:], wc[:])
            nc.sync.dma_start(wd_sb[:], wd[:])
            nc.sync.dma_start(w2h_sb[:], w2h[:])
            nc.sync.dma_start(w2l_sb[:], w2l[:])
            nc.sync.dma_start(w1u_sb[:], w1u[:])
            nc.sync.dma_start(b1_sb[:], b1[:])
            nc.sync.dma_start(b2_sb[:], b2[:])
            nc.sync.dma_start(sid_sb[:], sid[:])
            nc.sync.dma_start(uT_sb[:], uT[:])
            nc.sync.dma_start(idx_sb[:], idxw[:])

            # ---- one-time: v = u @ W1u, exact bf16 hi/lo split -> vtab DRAM ----
            # all 8 graph-chunks land in ONE psum bank ([128, 8*64] fp32 = 2KB)
            # so the whole table is produced by 8 MMs + 1 ACT + 2 DVE + 2 DMAs
            vtab = dram_pool.tile([N_GRAPHS, 4 * D], BF16)
            psv = psv_pool.tile([128, N_GRAPHS // 128 * D], F32, tag="psv")
            for k in range(N_GRAPHS // 128):
                nc.tensor.matmul(
                    psv[:, k * D : (k + 1) * D],
                    uT_sb[:, k * 128 : (k + 1) * 128],
                    w1u_sb[:],
                    start=True,
                    stop=True,
                )
            vhi_sb = vprep_pool.tile([128, N_GRAPHS // 128, D], BF16, tag="vhi_sb")
            vhi_f32 = vprep_pool.tile([128, N_GRAPHS // 128 * D], F32, tag="vhi_f32")
            vlo_sb = vprep_pool.tile([128, N_GRAPHS // 128, D], BF16, tag="vlo_sb")
            # hi = bf16(v)
            nc.scalar.activation(
                vhi_sb.rearrange("p a b -> p (a b)"),
                psv[:],
                mybir.ActivationFunctionType.Copy,
            )
            nc.vector.tensor_copy(vhi_f32[:], vhi_sb.rearrange("p a b -> p (a b)"))
            # lo = bf16(v - hi)
            nc.vector.tensor_sub(
                vlo_sb.rearrange("p a b -> p (a b)"), psv[:], vhi_f32[:]
            )
            # vtab[128k + p, 0:64] = hi chunk k, [64:128] = lo chunk k
            vtab_rows = vtab[:].rearrange("(a p) c -> p a c", p=128)
            nc.sync.dma_start(vtab_rows[:, :, 0:D], vhi_sb[:])
            nc.sync.dma_start(vtab_rows[:, :, D : 2 * D], vlo_sb[:])
            nc.sync.dma_start(vtab_rows[:, :, 2 * D : 3 * D], vhi_sb[:])
            nc.sync.dma_start(vtab_rows[:, :, 3 * D : 4 * D], vlo_sb[:])

            # ---- main loop over chunks of edges ----
            off = 0
            for csz in CHUNKS:
                gT = gath_pool.tile([128, 2, csz // 2], BF16, tag="gT")
                nc.gpsimd.dma_gather(
                    gT[:],
                    vtab[:],
                    idx_sb[:, off // 32 : (off + csz) // 32],
                    num_idxs=csz // 2,
                    num_idxs_reg=csz // 2,
                    elem_size=4 * D,
                    transpose=True,
                    single_packet=False,
                )

                t1c = dense_pool.tile([128, csz], BF16, tag="t1c")
                t2c = dense_pool.tile([128, csz], BF16, tag="t2c")
                t3c = dense_pool.tile([128, csz], BF16, tag="t3c")
                nc.sync.dma_start(t1c[:], p1[:, off : off + csz])
                nc.sync.dma_start(t2c[:], p2[:, off : off + csz])
                nc.sync.dma_start(t3c[:], p3[:, off : off + csz])

                oC = out_pool.tile([D, csz], F32, tag="oC")

                for t0 in range(0, csz, TS):
                    t1 = t0 + TS
                    psh = psh_pool.tile([D, TS], F32, tag="psh")
                    nc.tensor.matmul(
                        psh[:], wa_sb[:], t1c[:, t0:t1], start=True, stop=False
                    )
                    nc.tensor.matmul(
                        psh[:], wa_sb[:], t2c[:, t0:t1], start=False, stop=False
                    )
                    nc.tensor.matmul(
                        psh[:], wb_sb[:], t1c[:, t0:t1], start=False, stop=False
                    )
                    nc.tensor.matmul(
                        psh[:], wc_sb[:], t3c[:, t0:t1], start=False, stop=False
                    )
                    nc.tensor.matmul(
                        psh[:], wd_sb[:], t3c[0:D, t0:t1], start=False, stop=False
                    )
                    gb = t0 // (csz // 2)
                    gi = t0 % (csz // 2)
                    nc.tensor.matmul(
                        psh[:],
                        sid_sb[:],
                        gT[:, gb, gi : gi + TS],
                        start=False,
                        stop=True,
                    )
                    # h = relu(psh + b1); split into exact bf16 hi + lo
                    h_hi = h_pool.tile([D, TS], BF16, tag="h_hi")
                    nc.scalar.activation(
                        h_hi[:],
                        psh[:],
                        mybir.ActivationFunctionType.Relu,
                        bias=b1_sb[:],
                    )
                    h_f = h_pool.tile([D, TS], F32, tag="h_f")
                    nc.vector.tensor_scalar(
                        h_f[:],
                        psh[:],
                        b1_sb[:],
                        0.0,
                        op0=mybir.AluOpType.add,
                        op1=mybir.AluOpType.max,
                    )
                    h_lo = h_pool.tile([D, TS], BF16, tag="h_lo")
                    nc.vector.tensor_sub(h_lo[:], h_f[:], h_hi[:])

                    pso = pso_pool.tile([D, TS], F32, tag="pso")
                    nc.tensor.matmul(
                        pso[:], w2h_sb[:], h_hi[:], start=True, stop=False
                    )
                    nc.tensor.matmul(
                        pso[:], w2h_sb[:], h_lo[:], start=False, stop=False
                    )
                    nc.tensor.matmul(
                        pso[:], w2l_sb[:], h_hi[:], start=False, stop=True
                    )
                    nc.scalar.activation(
                        oC[:, t0:t1],
                        pso[:],
                        mybir.ActivationFunctionType.Relu,
                        bias=b2_sb[:],
                    )

                nc.sync.dma_start(outT[:, off : off + csz], oC[:])
                off += csz

    nc.compile()
    return nc


_NC = None


def get_nc():
    global _NC
    if _NC is None:
        _NC = build_nc()
    return _NC


def _hi_lo(x):
    """Exact bf16 hi/lo split of an fp32 array."""
    hi = x.astype(BF16_NP)
    lo = (x - hi.astype(np.float32)).astype(BF16_NP)
    return hi, lo


def prep_inputs(src, dest, edge_attr, u, batch, W1, b1, W2, b2):
    """Host-side shard + layout prep. Returns (in_maps, per-core device->orig maps).

    Edges are sorted by graph and padded so every graph has an even count;
    the device gathers one 512B table row per same-graph PAIR of edges
    (halving Q7 descriptor work). Within each chunk the device edge order is
    [even pair-members ; odd pair-members], matching the transposed-gather
    output layout [128, 2, csz/2]."""
    E_pad = N_CORES * EC
    batch = np.asarray(batch).astype(np.int64)

    counts = np.bincount(batch, minlength=N_GRAPHS)
    counts_p = counts + (counts & 1)
    total_p = int(counts_p.sum())
    assert total_p <= E_pad
    start_orig = np.concatenate([[0], np.cumsum(counts)[:-1]])
    start_p = np.concatenate([[0], np.cumsum(counts_p)[:-1]])
    order = np.argsort(batch, kind="stable")

    seq_e = np.full(E_pad, -1, np.int64)
    pos_in_graph = np.arange(N_EDGES) - np.repeat(start_orig, counts)
    seq_e[np.repeat(start_p, counts) + pos_in_graph] = order
    seq_g = np.zeros(E_pad, np.int16)
    seq_g[:total_p] = np.repeat(np.arange(N_GRAPHS, dtype=np.int16), counts_p)

    # device order within each chunk: even pair-members first, then odd
    perm = np.empty(EC, np.int64)
    offv = 0
    for csz in CHUNKS:
        h = csz // 2
        perm[offv : offv + h] = offv + 2 * np.arange(h)
        perm[offv + h : offv + csz] = offv + 1 + 2 * np.arange(h)
        offv += csz

    uT = np.ascontiguousarray(u.T.astype(np.float32))
    W1 = np.asarray(W1, dtype=np.float32)
    W2 = np.asarray(W2, dtype=np.float32)
    ws_hi, ws_lo = _hi_lo(W1[0:64])
    wd_hi, wd_lo = _hi_lo(W1[64:128])
    we_hi, we_lo = _hi_lo(W1[128:192])
    w2_hi, w2_lo = _hi_lo(W2)

    sid = np.zeros((128, D), dtype=BF16_NP)
    for m in range(D):
        sid[m, m] = 1.0
        sid[m + D, m] = 1.0

    common = {
        "uT": uT,
        "wa": np.vstack([ws_hi, wd_hi]),
        "wb": np.vstack([ws_lo, wd_lo]),
        "wc": np.vstack([we_hi, we_hi]),
        "wd": np.ascontiguousarray(we_lo),
        "w2h": np.ascontiguousarray(w2_hi),
        "w2l": np.ascontiguousarray(w2_lo),
        "w1u": np.ascontiguousarray(W1[192:256]),
        "b1": np.asarray(b1, dtype=np.float32).reshape(D, 1),
        "b2": np.asarray(b2, dtype=np.float32).reshape(D, 1),
        "sid": sid,
    }
    in_maps = []
    maps_c = []
    for c in range(N_CORES):
        ce = seq_e[c * EC : (c + 1) * EC]
        map_c = ce[perm]  # device position -> original edge id (-1 = pad)
        maps_c.append(map_c)
        valid = map_c >= 0
        mv = map_c[valid]
        pair_g = seq_g[c * EC : (c + 1) * EC][0::2]
        m = dict(common)
        m["idxw"] = np.ascontiguousarray(
            np.tile(pair_g.reshape(-1, 16).T, (8, 1))
        )
        def dev_hi_lo(x):
            xd = np.zeros((EC, D), np.float32)
            xd[valid] = x[mv]
            return _hi_lo(xd)
        s_hi, s_lo = dev_hi_lo(src)
        d_hi, d_lo = dev_hi_lo(dest)
        e_hi, e_lo = dev_hi_lo(edge_attr)
        m["p1"] = np.ascontiguousarray(np.vstack([s_hi.T, d_hi.T]))
        m["p2"] = np.ascontiguousarray(np.vstack([s_lo.T, d_lo.T]))
        m["p3"] = np.ascontiguousarray(np.vstack([e_hi.T, e_lo.T]))
        in_maps.append(m)
    return in_maps, maps_c


def kernel(src, dest, edge_attr, u, batch, W1, b1, W2, b2):
    global LAST_RESULTS
    nc = get_nc()
    in_maps, maps_c = prep_inputs(
        np.asarray(src, dtype=np.float32),
        np.asarray(dest, dtype=np.float32),
        np.asarray(edge_attr, dtype=np.float32),
        np.asarray(u, dtype=np.float32),
        np.asarray(batch),
        W1,
        b1,
        W2,
        b2,
    )
    res = run_bass_kernel_spmd(nc, in_maps, core_ids=list(range(N_CORES)), trace=TRACE)
    LAST_RESULTS = res
    out = np.empty((N_EDGES, D), dtype=np.float32)
    for c in range(N_CORES):
        out_dev = res.results[c]["outT"].T  # [EC, D], device order
        map_c = maps_c[c]
        valid = map_c >= 0
        out[map_c[valid]] = out_dev[valid]
    return out
